# revision 9
# baseline (speedup 1.0000x reference)
"""GCN (2-layer, PyG GCNConv semantics) on 8 Trainium2 NeuronCores.

Strategy (graph/data parallel, destination-bucketed, gather-based):
  - Nodes sorted by in-degree (desc) and dealt round-robin to the 8
    cores (6250 real + 150 pad each, 50 dest tiles of 128). Sorting
    makes each 128-node dest tile near-uniform in degree, so per-tile
    chunk counts K[t] (shared across cores, SPMD) are tight.
  - Normalization factored per-node: tables hold dinv[v]*h[v]; the
    aggregation is an unweighted sum; results are post-scaled by
    dinv[dst]. No per-edge multiplies.
  - Aggregation: per dest tile, dma_gather fetches, for each (dest
    partition p, chunk j) slot, the table rows [v, v+1] where v is the
    j-th in-neighbor of p (elem_size = 2 rows, elem_step = 1 row).
    Only the first half is consumed by the identity-matmul PSUM
    accumulation (partition index == destination, so scatter is free);
    the second row is don't-care. Indexing by node row keeps int16
    indices valid inside two OVERLAPPING 32768-row windows (bases 0
    and 18432); overlap-region edges are assigned to whichever window
    balances per-dest counts, minimizing padding. Pad slots point at
    guaranteed-zero pad rows. The same index arrays serve both layers.
  - Tables are bf16 (512B gather rows); accumulation stays fp32 in
    PSUM. Layer-2 features are padded 64->128 cols so the row stride
    meets the gather engine's 256B-multiple requirement.
  - Transformed tables are AllGathered (halo exchange) so every core
    gathers from a local full table; W1/W2 replicated.
"""

import numpy as np
import ml_dtypes

import concourse.bacc as bacc
import concourse.bass as bass
import concourse.mybir as mybir
import concourse.tile as tile
from concourse import bass_utils
from concourse.bass import ts
from concourse.masks import make_identity

N = 50000
F0, F1, F2 = 512, 128, 64
NCORES = 8
NSH = N // NCORES          # 6250 real nodes per core
NP = 6400                  # padded nodes per core (50 tiles of 128)
NT = NP // 128             # 50 dest tiles per core
TBL = NCORES * NP          # 51200 rows in the gathered tables
W1BASE = TBL - 32768       # 18432: window-1 base (windows overlap)
PAD0 = 6398                # core-0 pad row (all-zero), inside window 0
PAD1 = 51198 - W1BASE      # core-7 pad row, relative to window-1 base
F32 = mybir.dt.float32
BF16 = mybir.dt.bfloat16
I16 = mybir.dt.int16
BF = ml_dtypes.bfloat16

_TRACE = False
_LAST = None               # BassKernelResults of the most recent run


def _wrap16(flat_idx):
    """dma_gather index layout: element i at [i%16, i//16], replicated to
    128 partitions (one copy per GpSimd core)."""
    n = len(flat_idx)
    a = np.zeros((16, n // 16), np.int16)
    i = np.arange(n)
    a[i % 16, i // 16] = flat_idx.astype(np.int16)
    return np.tile(a, (8, 1))


def _host_prep(x, edge_index, W1, b1, W2, b2):
    src = np.asarray(edge_index[0], dtype=np.int64)
    dst = np.asarray(edge_index[1], dtype=np.int64)
    x = np.asarray(x, dtype=np.float32)

    deg = np.bincount(dst, minlength=N) + 1          # with self-loops
    order = np.argsort(-deg, kind="stable")          # rank -> node id
    r = np.arange(N)
    perm = np.empty(N, np.int64)                     # perm[c*NSH+pos] = node
    pos_of = np.empty(N, np.int64)                   # node -> c*NP + pos
    perm[(r % NCORES) * NSH + r // NCORES] = order
    pos_of[order] = (r % NCORES) * NP + r // NCORES

    all_src = np.concatenate([src, np.arange(N, dtype=np.int64)])
    all_dst = np.concatenate([dst, np.arange(N, dtype=np.int64)])
    dpos = pos_of[all_dst]
    spos = pos_of[all_src]

    # window assignment: rows < W1BASE forced to w0, >= 32768 forced to
    # w1, overlap region balances per-dest counts
    forced0 = spos < W1BASE
    flex = ~forced0 & (spos < 32768)
    E = len(all_src)
    f0 = np.bincount(dpos[forced0], minlength=TBL)
    fx = np.bincount(dpos[flex], minlength=TBL)
    tot = np.bincount(dpos, minlength=TBL)
    a0 = np.clip((tot + 1) // 2 - f0, 0, fx)         # flex edges -> w0

    eidx = np.arange(E)
    flex_e = eidx[flex]
    od = np.argsort(dpos[flex_e], kind="stable")
    fe = flex_e[od]
    fd = dpos[fe]
    starts = np.searchsorted(fd, np.arange(TBL))
    j_in_dest = np.arange(len(fe)) - starts[fd]
    in_w0 = forced0.copy()
    in_w0[fe[j_in_dest < a0[fd]]] = True

    def build_win(mask, base, pad_rel):
        dp = dpos[mask]
        sp = spos[mask] - base
        o = np.argsort(dp, kind="stable")
        sd = dp[o]
        ss = sp[o]
        st = np.searchsorted(sd, np.arange(TBL))
        j = np.arange(len(sd)) - st[sd]
        cnt = np.bincount(dp, minlength=TBL)
        Kct = cnt.reshape(NCORES, NT, 128).max(axis=2)
        K = Kct.max(axis=0).astype(np.int64)
        offs = np.concatenate([[0], np.cumsum(K)]).astype(np.int64)
        idx = np.full((NCORES, 128, int(offs[-1])), pad_rel, dtype=np.int64)
        c_of = sd // NP
        lp = sd % NP
        idx[c_of, lp % 128, offs[lp // 128] + j] = ss
        return K, offs, idx

    K0, offs0, idx0 = build_win(in_w0, 0, PAD0)
    K1, offs1, idx1 = build_win(~in_w0, W1BASE, PAD1)

    def wrap_core(idx_c, K, offs):
        blocks = []
        for t in range(NT):
            if K[t] == 0:
                continue
            blk = idx_c[:, offs[t]:offs[t + 1]]       # [128, K[t]]
            flat = blk.T.reshape(-1)                  # i = j*128 + p
            blocks.append(_wrap16(flat))
        if not blocks:
            return np.zeros((128, 1), np.int16)
        return np.ascontiguousarray(np.concatenate(blocks, axis=1))

    in_maps = []
    for c in range(NCORES):
        pc = perm[c * NSH:(c + 1) * NSH]
        xp = np.zeros((NP, F0), dtype=np.float32)
        xp[:NSH] = x[pc]
        xt4 = np.ascontiguousarray(xp.T.reshape(4, 128, NP)).astype(BF)
        degp = np.ones(NP, dtype=np.float32)
        degp[:NSH] = deg[pc].astype(np.float32)
        degT = np.ascontiguousarray(degp.reshape(NT, 128).T)  # [128, NT]
        m = {
            "xt4": xt4,
            "w1r": np.ascontiguousarray(
                np.asarray(W1, np.float32).reshape(4, 128, F1)).astype(BF),
            "w2": np.ascontiguousarray(np.asarray(W2, np.float32)).astype(BF),
            "degT": degT,
            "idx0": wrap_core(idx0[c], K0, offs0),
            "idx1": wrap_core(idx1[c], K1, offs1),
        }
        in_maps.append(m)

    return in_maps, perm, K0, K1


def _build(K0, K1, w0n, w1n, b1_zero, b2_zero):
    Relu = mybir.ActivationFunctionType.Relu
    Copy = mybir.ActivationFunctionType.Copy
    Sqrt = mybir.ActivationFunctionType.Sqrt

    nc = bacc.Bacc("TRN2", target_bir_lowering=False, num_devices=NCORES)

    xt4_d = nc.dram_tensor("xt4", [4, 128, NP], BF16, kind="ExternalInput")
    w1r_d = nc.dram_tensor("w1r", [4, 128, F1], BF16, kind="ExternalInput")
    w2_d = nc.dram_tensor("w2", [F1, F2], BF16, kind="ExternalInput")
    degT_d = nc.dram_tensor("degT", [128, NT], F32, kind="ExternalInput")
    i0_d = nc.dram_tensor("idx0", [128, w0n], I16, kind="ExternalInput")
    i1_d = nc.dram_tensor("idx1", [128, w1n], I16, kind="ExternalInput")
    b1r_d = b2r_d = None
    if not b1_zero:
        b1r_d = nc.dram_tensor("b1r", [128, F1], F32, kind="ExternalInput")
    if not b2_zero:
        b2r_d = nc.dram_tensor("b2r", [128, F2], F32, kind="ExternalInput")
    out_d = nc.dram_tensor("out", [NP, F2], F32, kind="ExternalOutput")

    h1_loc = nc.dram_tensor("h1_loc", [NP, F1], BF16, kind="Internal")
    h1_full = nc.dram_tensor("h1_full", [TBL + 2, F1], BF16, kind="Internal",
                             addr_space="Shared")
    h2_loc = nc.dram_tensor("h2_loc", [NP, 128], BF16, kind="Internal")
    h2_full = nc.dram_tensor("h2_full", [TBL + 2, 128], BF16, kind="Internal",
                             addr_space="Shared")

    rg = [list(range(NCORES))]
    K = (np.asarray(K0) + np.asarray(K1)).astype(np.int64)
    KMAX = int(K.max())
    GKMAX = int(max(K[t0:t0 + 2].sum() for t0 in range(0, NT, 2)))
    GROUPS = []
    cur, csum = [], 0
    for t in range(NT):
        kt = int(K[t])
        if kt == 0:
            continue
        if cur and csum + kt > GKMAX:
            GROUPS.append(cur)
            cur, csum = [], 0
        cur.append(t)
        csum += kt
    if cur:
        GROUPS.append(cur)

    with tile.TileContext(nc, num_cores=NCORES) as tc:
        with (
            tc.tile_pool(name="const", bufs=1) as cpool,
            tc.tile_pool(name="out1", bufs=1) as o1pool,
            tc.tile_pool(name="stream", bufs=3) as spool,
            tc.tile_pool(name="msg", bufs=2) as mpool,
            tc.tile_pool(name="psA", bufs=2, space="PSUM") as psA,
            tc.tile_pool(name="psE", bufs=1, space="PSUM") as psE,
            tc.tile_pool(name="psG", bufs=2, space="PSUM") as psG,
        ):
            # ---- constants -------------------------------------------------
            w1sb = cpool.tile([128, 4, F1], BF16)
            nc.sync.dma_start(out=w1sb[:], in_=w1r_d[:].rearrange("k p f -> p k f"))
            w2sb = cpool.tile([128, F2], BF16)
            nc.sync.dma_start(out=w2sb[:], in_=w2_d[:])
            degsb = cpool.tile([128, NT], F32)
            nc.sync.dma_start(out=degsb[:], in_=degT_d[:])
            i0sb = cpool.tile([128, w0n], I16)
            nc.sync.dma_start(out=i0sb[:], in_=i0_d[:])
            i1sb = cpool.tile([128, w1n], I16)
            nc.sync.dma_start(out=i1sb[:], in_=i1_d[:])
            identf = cpool.tile([128, 128], F32)
            make_identity(nc, identf[:])
            identb = cpool.tile([128, 128], BF16)
            nc.scalar.copy(identb[:], identf[:])
            b1sb = b2sb = None
            if not b1_zero:
                b1sb = cpool.tile([128, F1], F32)
                nc.sync.dma_start(out=b1sb[:], in_=b1r_d[:])
            if not b2_zero:
                b2sb = cpool.tile([128, F2], F32)
                nc.sync.dma_start(out=b2sb[:], in_=b2r_d[:])

            rec = cpool.tile([128, NT], F32)
            nc.vector.reciprocal(rec[:], degsb[:])
            dinv = cpool.tile([128, NT], F32)
            nc.scalar.activation(dinv[:], rec[:], Sqrt)

            out1 = o1pool.tile([128, NT, F1], BF16)

            # ---- phase B: h1 = (x @ W1) * dinv -----------------------------
            for t in range(NT):
                xt = spool.tile([128, 4, 128], BF16, tag="xt")
                nc.sync.dma_start(
                    out=xt[:],
                    in_=xt4_d[:, :, ts(t, 128)].rearrange("k p n -> p k n"))
                ph = psA.tile([128, F1], F32, tag="ph")
                for k in range(4):
                    nc.tensor.matmul(ph[:], lhsT=xt[:, k, :], rhs=w1sb[:, k, :],
                                     start=(k == 0), stop=(k == 3))
                h1t = spool.tile([128, F1], BF16, tag="h1t")
                nc.scalar.activation(h1t[:], ph[:], Copy, scale=dinv[:, t:t + 1])
                nc.sync.dma_start(out=h1_loc[ts(t, 128), :], in_=h1t[:])

            # ---- phase C: AllGather h1 -------------------------------------
            nc.gpsimd.collective_compute(
                "AllGather", mybir.AluOpType.bypass, replica_groups=rg,
                ins=[h1_loc[:]], outs=[h1_full[0:TBL, :]])

            # ---- gather windows: overlapping [v, v+1] row-pair reads -------
            def win_ap(table, row_elems, base, nrows):
                a = table[base:base + nrows, :]
                return bass.AP(a.tensor, a.offset,
                               [[row_elems, nrows], [1, 2 * row_elems]])

            # ---- aggregation helper (G tiles batched per gather pair) ------
            def aggregate(table, row_elems, feat, ps_pool, ps_tag, consume):
                o0 = o1_ = 0
                ap0 = win_ap(table, row_elems, 0, 32768)
                ap1 = win_ap(table, row_elems, W1BASE, TBL - W1BASE)
                for g in GROUPS:
                    k0g = sum(int(K0[t]) for t in g)
                    k1g = sum(int(K1[t]) for t in g)
                    msg = mpool.tile([128, GKMAX, 2 * row_elems], BF16,
                                     tag="msg")
                    if k0g:
                        n0 = 128 * k0g
                        nc.gpsimd.dma_gather(
                            out_ap=msg[:, :k0g, :], in_ap=ap0,
                            idxs_ap=i0sb[:, o0:o0 + n0 // 16],
                            num_idxs=n0, num_idxs_reg=n0,
                            elem_size=2 * row_elems, elem_step=row_elems,
                            single_packet=False)
                        o0 += n0 // 16
                    if k1g:
                        n1 = 128 * k1g
                        nc.gpsimd.dma_gather(
                            out_ap=msg[:, k0g:k0g + k1g, :], in_ap=ap1,
                            idxs_ap=i1sb[:, o1_:o1_ + n1 // 16],
                            num_idxs=n1, num_idxs_reg=n1,
                            elem_size=2 * row_elems, elem_step=row_elems,
                            single_packet=False)
                        o1_ += n1 // 16
                    c0 = 0
                    c1 = k0g
                    for t in g:
                        k0, k1 = int(K0[t]), int(K1[t])
                        po = ps_pool.tile([128, feat], F32, tag=ps_tag)
                        nk = k0 + k1
                        for j in range(nk):
                            col = c0 + j if j < k0 else c1 + (j - k0)
                            nc.tensor.matmul(po[:], lhsT=identb[:],
                                             rhs=msg[:, col, 0:feat],
                                             start=(j == 0),
                                             stop=(j == nk - 1))
                        c0 += k0
                        c1 += k1
                        consume(t, po)

            # ---- phase D+E: out1 = relu(dinv*Agg(h1)); h2 = (out1@W2)*dinv -
            def consume1(t, po):
                if b1_zero:
                    nc.scalar.activation(out1[:, t, :], po[:], Relu,
                                         scale=dinv[:, t:t + 1])
                else:
                    tmp = spool.tile([128, F1], F32, tag="tmp1")
                    nc.scalar.activation(tmp[:], po[:], Copy,
                                         scale=dinv[:, t:t + 1])
                    tmp2 = spool.tile([128, F1], F32, tag="tmp2")
                    nc.vector.tensor_tensor(out=tmp2[:], in0=tmp[:],
                                            in1=b1sb[:],
                                            op=mybir.AluOpType.add)
                    nc.scalar.activation(out1[:, t, :], tmp2[:], Relu)
                phase_e(t)

            def phase_e(t):
                pT = psE.tile([128, F1], BF16, tag="pT")
                nc.tensor.transpose(pT[:], out1[:, t, :], identb[:])
                o1T = spool.tile([128, F1], BF16, tag="o1T")
                nc.scalar.copy(o1T[:], pT[:])
                ph2 = psE.tile([128, F2], F32, tag="ph2")
                nc.tensor.matmul(ph2[:], lhsT=o1T[:], rhs=w2sb[:],
                                 start=True, stop=True)
                h2t = spool.tile([128, 128], BF16, tag="h2t")
                nc.vector.memset(h2t[:, F2:128], 0.0)
                nc.scalar.activation(h2t[:, 0:F2], ph2[:], Copy,
                                     scale=dinv[:, t:t + 1])
                nc.sync.dma_start(out=h2_loc[ts(t, 128), :], in_=h2t[:])

            # zero-degree (all-pad) tiles still need zero h2 rows
            for t in range(NT):
                if int(K[t]) == 0:
                    nc.vector.memset(out1[:, t, :], 0.0)
                    phase_e(t)

            aggregate(h1_full, F1, F1, psA, "po", consume1)

            # ---- phase F: AllGather h2 -------------------------------------
            nc.gpsimd.collective_compute(
                "AllGather", mybir.AluOpType.bypass, replica_groups=rg,
                ins=[h2_loc[:]], outs=[h2_full[0:TBL, :]])

            # ---- phase G: out = dinv * Agg(h2) + b2 ------------------------
            def consume2(t, po):
                o2t = spool.tile([128, F2], F32, tag="o2t")
                nc.scalar.activation(o2t[:], po[:], Copy,
                                     scale=dinv[:, t:t + 1])
                if not b2_zero:
                    nc.vector.tensor_tensor(out=o2t[:], in0=o2t[:],
                                            in1=b2sb[:],
                                            op=mybir.AluOpType.add)
                nc.sync.dma_start(out=out_d[ts(t, 128), :], in_=o2t[:])

            aggregate(h2_full, 128, F2, psG, "po2", consume2)

    nc.compile()
    return nc


def kernel(x, edge_index, W1, b1, W2, b2):
    global _LAST
    b1 = np.asarray(b1, np.float32)
    b2 = np.asarray(b2, np.float32)
    in_maps, perm, K0, K1 = _host_prep(x, edge_index, W1, b1, W2, b2)

    b1_zero = bool(np.all(b1 == 0))
    b2_zero = bool(np.all(b2 == 0))
    if not b1_zero:
        for m in in_maps:
            m["b1r"] = np.ascontiguousarray(np.tile(b1[None, :], (128, 1)))
    if not b2_zero:
        for m in in_maps:
            m["b2r"] = np.ascontiguousarray(np.tile(b2[None, :], (128, 1)))

    w0n = in_maps[0]["idx0"].shape[1]
    w1n = in_maps[0]["idx1"].shape[1]
    nc = _build(K0, K1, w0n, w1n, b1_zero, b2_zero)

    res = bass_utils.run_bass_kernel_spmd(
        nc, in_maps, core_ids=list(range(NCORES)), trace=_TRACE)
    _LAST = res

    out = np.empty((N, F2), dtype=np.float32)
    for c in range(NCORES):
        pc = perm[c * NSH:(c + 1) * NSH]
        out[pc] = res.results[c]["out"][:NSH]
    return out


# revision 10
# speedup vs baseline: 1.0121x; 1.0121x over previous
"""GCN (2-layer, PyG GCNConv semantics) on 8 Trainium2 NeuronCores.

Strategy (graph/data parallel, destination-bucketed, gather-based):
  - Nodes sorted by in-degree (desc) and dealt round-robin to the 8
    cores (6250 real + 150 pad each, 50 dest tiles of 128). Sorting
    makes each 128-node dest tile near-uniform in degree, so per-tile
    chunk counts K[t] (shared across cores, SPMD) are tight.
  - Normalization factored per-node: tables hold dinv[v]*h[v]; the
    aggregation is an unweighted sum; results are post-scaled by
    dinv[dst]. No per-edge multiplies.
  - Aggregation: per dest tile, dma_gather fetches, for each (dest
    partition p, chunk j) slot, the table rows [v, v+1] where v is the
    j-th in-neighbor of p (elem_size = 2 rows, elem_step = 1 row).
    Only the first half is consumed by the identity-matmul PSUM
    accumulation (partition index == destination, so scatter is free);
    the second row is don't-care. Indexing by node row keeps int16
    indices valid inside two OVERLAPPING 32768-row windows (bases 0
    and 18432); overlap-region edges are assigned to whichever window
    balances per-dest counts, minimizing padding. Pad slots point at
    guaranteed-zero pad rows. The same index arrays serve both layers.
  - Tables are bf16 (512B gather rows); accumulation stays fp32 in
    PSUM. Layer-2 features are padded 64->128 cols so the row stride
    meets the gather engine's 256B-multiple requirement.
  - Transformed tables are AllGathered (halo exchange) so every core
    gathers from a local full table; W1/W2 replicated.
"""

import numpy as np
import ml_dtypes

import concourse.bacc as bacc
import concourse.bass as bass
import concourse.mybir as mybir
import concourse.tile as tile
from concourse import bass_utils
from concourse.bass import ts
from concourse.masks import make_identity

N = 50000
F0, F1, F2 = 512, 128, 64
NCORES = 8
NSH = N // NCORES          # 6250 real nodes per core
NP = 6400                  # padded nodes per core (50 tiles of 128)
NT = NP // 128             # 50 dest tiles per core
TBL = NCORES * NP          # 51200 rows in the gathered tables
W1BASE = TBL - 32768       # 18432: window-1 base (windows overlap)
PAD0 = 6398                # core-0 pad row (all-zero), inside window 0
PAD1 = 51198 - W1BASE      # core-7 pad row, relative to window-1 base
F32 = mybir.dt.float32
BF16 = mybir.dt.bfloat16
I16 = mybir.dt.int16
BF = ml_dtypes.bfloat16

_TRACE = False
_LAST = None               # BassKernelResults of the most recent run


def _wrap16(flat_idx):
    """dma_gather index layout: element i at [i%16, i//16], replicated to
    128 partitions (one copy per GpSimd core)."""
    n = len(flat_idx)
    a = np.zeros((16, n // 16), np.int16)
    i = np.arange(n)
    a[i % 16, i // 16] = flat_idx.astype(np.int16)
    return np.tile(a, (8, 1))


def _host_prep(x, edge_index, W1, b1, W2, b2):
    src = np.asarray(edge_index[0], dtype=np.int64)
    dst = np.asarray(edge_index[1], dtype=np.int64)
    x = np.asarray(x, dtype=np.float32)

    deg = np.bincount(dst, minlength=N) + 1          # with self-loops
    order = np.argsort(-deg, kind="stable")          # rank -> node id
    r = np.arange(N)
    perm = np.empty(N, np.int64)                     # perm[c*NSH+pos] = node
    pos_of = np.empty(N, np.int64)                   # node -> c*NP + pos
    perm[(r % NCORES) * NSH + r // NCORES] = order
    pos_of[order] = (r % NCORES) * NP + r // NCORES

    all_src = np.concatenate([src, np.arange(N, dtype=np.int64)])
    all_dst = np.concatenate([dst, np.arange(N, dtype=np.int64)])
    dpos = pos_of[all_dst]
    spos = pos_of[all_src]

    # window assignment: rows < W1BASE forced to w0, >= 32768 forced to
    # w1, overlap region balances per-dest counts
    forced0 = spos < W1BASE
    flex = ~forced0 & (spos < 32768)
    E = len(all_src)
    f0 = np.bincount(dpos[forced0], minlength=TBL)
    fx = np.bincount(dpos[flex], minlength=TBL)
    tot = np.bincount(dpos, minlength=TBL)
    a0 = np.clip((tot + 1) // 2 - f0, 0, fx)         # flex edges -> w0

    eidx = np.arange(E)
    flex_e = eidx[flex]
    od = np.argsort(dpos[flex_e], kind="stable")
    fe = flex_e[od]
    fd = dpos[fe]
    starts = np.searchsorted(fd, np.arange(TBL))
    j_in_dest = np.arange(len(fe)) - starts[fd]
    in_w0 = forced0.copy()
    in_w0[fe[j_in_dest < a0[fd]]] = True

    def build_win(mask, base, pad_rel):
        dp = dpos[mask]
        sp = spos[mask] - base
        o = np.argsort(dp, kind="stable")
        sd = dp[o]
        ss = sp[o]
        st = np.searchsorted(sd, np.arange(TBL))
        j = np.arange(len(sd)) - st[sd]
        cnt = np.bincount(dp, minlength=TBL)
        Kct = cnt.reshape(NCORES, NT, 128).max(axis=2)
        K = Kct.max(axis=0).astype(np.int64)
        offs = np.concatenate([[0], np.cumsum(K)]).astype(np.int64)
        idx = np.full((NCORES, 128, int(offs[-1])), pad_rel, dtype=np.int64)
        c_of = sd // NP
        lp = sd % NP
        idx[c_of, lp % 128, offs[lp // 128] + j] = ss
        return K, offs, idx

    K0, offs0, idx0 = build_win(in_w0, 0, PAD0)
    K1, offs1, idx1 = build_win(~in_w0, W1BASE, PAD1)

    def wrap_core(idx_c, K, offs):
        blocks = []
        for t in range(NT):
            if K[t] == 0:
                continue
            blk = idx_c[:, offs[t]:offs[t + 1]]       # [128, K[t]]
            flat = blk.T.reshape(-1)                  # i = j*128 + p
            blocks.append(_wrap16(flat))
        if not blocks:
            return np.zeros((128, 1), np.int16)
        return np.ascontiguousarray(np.concatenate(blocks, axis=1))

    in_maps = []
    for c in range(NCORES):
        pc = perm[c * NSH:(c + 1) * NSH]
        xp = np.zeros((NP, F0), dtype=np.float32)
        xp[:NSH] = x[pc]
        xt4 = np.ascontiguousarray(xp.T.reshape(4, 128, NP)).astype(BF)
        degp = np.ones(NP, dtype=np.float32)
        degp[:NSH] = deg[pc].astype(np.float32)
        degT = np.ascontiguousarray(degp.reshape(NT, 128).T)  # [128, NT]
        m = {
            "xt4": xt4,
            "w1r": np.ascontiguousarray(
                np.asarray(W1, np.float32).reshape(4, 128, F1)).astype(BF),
            "w2": np.ascontiguousarray(np.asarray(W2, np.float32)).astype(BF),
            "degT": degT,
            "idx0": wrap_core(idx0[c], K0, offs0),
            "idx1": wrap_core(idx1[c], K1, offs1),
        }
        in_maps.append(m)

    return in_maps, perm, K0, K1


def _build(K0, K1, w0n, w1n, b1_zero, b2_zero):
    Relu = mybir.ActivationFunctionType.Relu
    Copy = mybir.ActivationFunctionType.Copy
    Sqrt = mybir.ActivationFunctionType.Sqrt

    nc = bacc.Bacc("TRN2", target_bir_lowering=False, num_devices=NCORES)

    xt4_d = nc.dram_tensor("xt4", [4, 128, NP], BF16, kind="ExternalInput")
    w1r_d = nc.dram_tensor("w1r", [4, 128, F1], BF16, kind="ExternalInput")
    w2_d = nc.dram_tensor("w2", [F1, F2], BF16, kind="ExternalInput")
    degT_d = nc.dram_tensor("degT", [128, NT], F32, kind="ExternalInput")
    i0_d = nc.dram_tensor("idx0", [128, w0n], I16, kind="ExternalInput")
    i1_d = nc.dram_tensor("idx1", [128, w1n], I16, kind="ExternalInput")
    b1r_d = b2r_d = None
    if not b1_zero:
        b1r_d = nc.dram_tensor("b1r", [128, F1], F32, kind="ExternalInput")
    if not b2_zero:
        b2r_d = nc.dram_tensor("b2r", [128, F2], F32, kind="ExternalInput")
    out_d = nc.dram_tensor("out", [NP, F2], F32, kind="ExternalOutput")

    h1_loc = nc.dram_tensor("h1_loc", [NP, F1], BF16, kind="Internal")
    h1_full = nc.dram_tensor("h1_full", [TBL + 2, F1], BF16, kind="Internal",
                             addr_space="Shared")
    h2_loc = nc.dram_tensor("h2_loc", [NP, 128], BF16, kind="Internal")
    h2_full = nc.dram_tensor("h2_full", [TBL + 2, 128], BF16, kind="Internal",
                             addr_space="Shared")

    rg = [list(range(NCORES))]
    K = (np.asarray(K0) + np.asarray(K1)).astype(np.int64)
    KMAX = int(K.max())
    GKMAX = int(max(K[t0:t0 + 2].sum() for t0 in range(0, NT, 2)))
    GROUPS = []
    for t0 in range(0, NT, 2):
        g = [t for t in range(t0, min(t0 + 2, NT)) if int(K[t]) > 0]
        if g:
            GROUPS.append(g)

    with tile.TileContext(nc, num_cores=NCORES) as tc:
        with (
            tc.tile_pool(name="const", bufs=1) as cpool,
            tc.tile_pool(name="out1", bufs=1) as o1pool,
            tc.tile_pool(name="stream", bufs=3) as spool,
            tc.tile_pool(name="msg", bufs=2) as mpool,
            tc.tile_pool(name="psA", bufs=2, space="PSUM") as psA,
            tc.tile_pool(name="psE", bufs=1, space="PSUM") as psE,
            tc.tile_pool(name="psG", bufs=2, space="PSUM") as psG,
        ):
            # ---- constants -------------------------------------------------
            w1sb = cpool.tile([128, 4, F1], BF16)
            nc.sync.dma_start(out=w1sb[:], in_=w1r_d[:].rearrange("k p f -> p k f"))
            w2sb = cpool.tile([128, F2], BF16)
            nc.sync.dma_start(out=w2sb[:], in_=w2_d[:])
            degsb = cpool.tile([128, NT], F32)
            nc.sync.dma_start(out=degsb[:], in_=degT_d[:])
            i0sb = cpool.tile([128, w0n], I16)
            nc.sync.dma_start(out=i0sb[:], in_=i0_d[:])
            i1sb = cpool.tile([128, w1n], I16)
            nc.sync.dma_start(out=i1sb[:], in_=i1_d[:])
            identf = cpool.tile([128, 128], F32)
            make_identity(nc, identf[:])
            identb = cpool.tile([128, 128], BF16)
            nc.scalar.copy(identb[:], identf[:])
            b1sb = b2sb = None
            if not b1_zero:
                b1sb = cpool.tile([128, F1], F32)
                nc.sync.dma_start(out=b1sb[:], in_=b1r_d[:])
            if not b2_zero:
                b2sb = cpool.tile([128, F2], F32)
                nc.sync.dma_start(out=b2sb[:], in_=b2r_d[:])

            rec = cpool.tile([128, NT], F32)
            nc.vector.reciprocal(rec[:], degsb[:])
            dinv = cpool.tile([128, NT], F32)
            nc.scalar.activation(dinv[:], rec[:], Sqrt)

            out1 = o1pool.tile([128, NT, F1], BF16)

            # ---- phase B: h1 = (x @ W1) * dinv -----------------------------
            for t in range(NT):
                xt = spool.tile([128, 4, 128], BF16, tag="xt")
                nc.sync.dma_start(
                    out=xt[:],
                    in_=xt4_d[:, :, ts(t, 128)].rearrange("k p n -> p k n"))
                ph = psA.tile([128, F1], F32, tag="ph")
                for k in range(4):
                    nc.tensor.matmul(ph[:], lhsT=xt[:, k, :], rhs=w1sb[:, k, :],
                                     start=(k == 0), stop=(k == 3))
                h1t = spool.tile([128, F1], BF16, tag="h1t")
                nc.scalar.activation(h1t[:], ph[:], Copy, scale=dinv[:, t:t + 1])
                nc.sync.dma_start(out=h1_loc[ts(t, 128), :], in_=h1t[:])

            # ---- phase C: AllGather h1 -------------------------------------
            nc.gpsimd.collective_compute(
                "AllGather", mybir.AluOpType.bypass, replica_groups=rg,
                ins=[h1_loc[:]], outs=[h1_full[0:TBL, :]])

            # ---- gather windows: overlapping [v, v+1] row-pair reads -------
            def win_ap(table, row_elems, base, nrows):
                a = table[base:base + nrows, :]
                return bass.AP(a.tensor, a.offset,
                               [[row_elems, nrows], [1, 2 * row_elems]])

            # ---- aggregation helper (G tiles batched per gather pair) ------
            def aggregate(table, row_elems, feat, ps_pool, ps_tag, consume):
                o0 = o1_ = 0
                ap0 = win_ap(table, row_elems, 0, 32768)
                ap1 = win_ap(table, row_elems, W1BASE, TBL - W1BASE)
                for g in GROUPS:
                    k0g = sum(int(K0[t]) for t in g)
                    k1g = sum(int(K1[t]) for t in g)
                    msg = mpool.tile([128, GKMAX, 2 * row_elems], BF16,
                                     tag="msg")
                    if k0g:
                        n0 = 128 * k0g
                        nc.gpsimd.dma_gather(
                            out_ap=msg[:, :k0g, :], in_ap=ap0,
                            idxs_ap=i0sb[:, o0:o0 + n0 // 16],
                            num_idxs=n0, num_idxs_reg=n0,
                            elem_size=2 * row_elems, elem_step=row_elems,
                            single_packet=False)
                        o0 += n0 // 16
                    if k1g:
                        n1 = 128 * k1g
                        nc.gpsimd.dma_gather(
                            out_ap=msg[:, k0g:k0g + k1g, :], in_ap=ap1,
                            idxs_ap=i1sb[:, o1_:o1_ + n1 // 16],
                            num_idxs=n1, num_idxs_reg=n1,
                            elem_size=2 * row_elems, elem_step=row_elems,
                            single_packet=False)
                        o1_ += n1 // 16
                    c0 = 0
                    c1 = k0g
                    for t in g:
                        k0, k1 = int(K0[t]), int(K1[t])
                        po = ps_pool.tile([128, feat], F32, tag=ps_tag)
                        nk = k0 + k1
                        for j in range(nk):
                            col = c0 + j if j < k0 else c1 + (j - k0)
                            nc.tensor.matmul(po[:], lhsT=identb[:],
                                             rhs=msg[:, col, 0:feat],
                                             start=(j == 0),
                                             stop=(j == nk - 1))
                        c0 += k0
                        c1 += k1
                        consume(t, po)

            # ---- phase D+E: out1 = relu(dinv*Agg(h1)); h2 = (out1@W2)*dinv -
            def consume1(t, po):
                if b1_zero:
                    nc.scalar.activation(out1[:, t, :], po[:], Relu,
                                         scale=dinv[:, t:t + 1])
                else:
                    tmp = spool.tile([128, F1], F32, tag="tmp1")
                    nc.scalar.activation(tmp[:], po[:], Copy,
                                         scale=dinv[:, t:t + 1])
                    tmp2 = spool.tile([128, F1], F32, tag="tmp2")
                    nc.vector.tensor_tensor(out=tmp2[:], in0=tmp[:],
                                            in1=b1sb[:],
                                            op=mybir.AluOpType.add)
                    nc.scalar.activation(out1[:, t, :], tmp2[:], Relu)
                phase_e(t)

            def phase_e(t):
                pT = psE.tile([128, F1], BF16, tag="pT")
                nc.tensor.transpose(pT[:], out1[:, t, :], identb[:])
                o1T = spool.tile([128, F1], BF16, tag="o1T")
                nc.scalar.copy(o1T[:], pT[:])
                ph2 = psE.tile([128, F2], F32, tag="ph2")
                nc.tensor.matmul(ph2[:], lhsT=o1T[:], rhs=w2sb[:],
                                 start=True, stop=True)
                h2t = spool.tile([128, 128], BF16, tag="h2t")
                nc.vector.memset(h2t[:, F2:128], 0.0)
                nc.scalar.activation(h2t[:, 0:F2], ph2[:], Copy,
                                     scale=dinv[:, t:t + 1])
                nc.sync.dma_start(out=h2_loc[ts(t, 128), :], in_=h2t[:])

            # zero-degree (all-pad) tiles still need zero h2 rows
            for t in range(NT):
                if int(K[t]) == 0:
                    nc.vector.memset(out1[:, t, :], 0.0)
                    phase_e(t)

            aggregate(h1_full, F1, F1, psA, "po", consume1)

            # ---- phase F: AllGather h2 -------------------------------------
            nc.gpsimd.collective_compute(
                "AllGather", mybir.AluOpType.bypass, replica_groups=rg,
                ins=[h2_loc[:]], outs=[h2_full[0:TBL, :]])

            # ---- phase G: out = dinv * Agg(h2) + b2 ------------------------
            def consume2(t, po):
                o2t = spool.tile([128, F2], F32, tag="o2t")
                nc.scalar.activation(o2t[:], po[:], Copy,
                                     scale=dinv[:, t:t + 1])
                if not b2_zero:
                    nc.vector.tensor_tensor(out=o2t[:], in0=o2t[:],
                                            in1=b2sb[:],
                                            op=mybir.AluOpType.add)
                nc.sync.dma_start(out=out_d[ts(t, 128), :], in_=o2t[:])

            aggregate(h2_full, 128, F2, psG, "po2", consume2)

    nc.compile()
    return nc


def kernel(x, edge_index, W1, b1, W2, b2):
    global _LAST
    b1 = np.asarray(b1, np.float32)
    b2 = np.asarray(b2, np.float32)
    in_maps, perm, K0, K1 = _host_prep(x, edge_index, W1, b1, W2, b2)

    b1_zero = bool(np.all(b1 == 0))
    b2_zero = bool(np.all(b2 == 0))
    if not b1_zero:
        for m in in_maps:
            m["b1r"] = np.ascontiguousarray(np.tile(b1[None, :], (128, 1)))
    if not b2_zero:
        for m in in_maps:
            m["b2r"] = np.ascontiguousarray(np.tile(b2[None, :], (128, 1)))

    w0n = in_maps[0]["idx0"].shape[1]
    w1n = in_maps[0]["idx1"].shape[1]
    nc = _build(K0, K1, w0n, w1n, b1_zero, b2_zero)

    res = bass_utils.run_bass_kernel_spmd(
        nc, in_maps, core_ids=list(range(NCORES)), trace=_TRACE)
    _LAST = res

    out = np.empty((N, F2), dtype=np.float32)
    for c in range(NCORES):
        pc = perm[c * NSH:(c + 1) * NSH]
        out[pc] = res.results[c]["out"][:NSH]
    return out


# revision 13
# speedup vs baseline: 1.0483x; 1.0358x over previous
"""GCN (2-layer, PyG GCNConv semantics) on 8 Trainium2 NeuronCores.

Strategy (graph/data parallel, destination-bucketed, gather-based):
  - Nodes sorted by in-degree (desc) and dealt round-robin to the 8
    cores (6250 real + 150 pad each, 50 dest tiles of 128). Sorting
    makes each 128-node dest tile near-uniform in degree, so per-tile
    chunk counts K[t] (shared across cores, SPMD) are tight.
  - Normalization factored per-node: tables hold dinv[v]*h[v]; the
    aggregation is an unweighted sum; results are post-scaled by
    dinv[dst]. No per-edge multiplies.
  - Aggregation: per dest tile, dma_gather fetches, for each (dest
    partition p, chunk j) slot, the table rows [v, v+1] where v is the
    j-th in-neighbor of p (elem_size = 2 rows, elem_step = 1 row).
    Only the first half is consumed by the identity-matmul PSUM
    accumulation (partition index == destination, so scatter is free);
    the second row is don't-care. Indexing by node row keeps int16
    indices valid inside two OVERLAPPING 32768-row windows (bases 0
    and 18432); overlap-region edges are assigned to whichever window
    balances per-dest counts, minimizing padding. Pad slots point at
    guaranteed-zero pad rows. The same index arrays serve both layers.
  - Tables are bf16 (512B gather rows); accumulation stays fp32 in
    PSUM. Layer-2 features are padded 64->128 cols so the row stride
    meets the gather engine's 256B-multiple requirement.
  - Transformed tables are AllGathered (halo exchange) so every core
    gathers from a local full table; W1/W2 replicated.
"""

import numpy as np
import ml_dtypes

import concourse.bacc as bacc
import concourse.bass as bass
import concourse.mybir as mybir
import concourse.tile as tile
from concourse import bass_utils
from concourse.bass import ts
from concourse.masks import make_identity

N = 50000
F0, F1, F2 = 512, 128, 64
NCORES = 8
NSH = N // NCORES          # 6250 real nodes per core
NP = 6400                  # padded nodes per core (50 tiles of 128)
NT = NP // 128             # 50 dest tiles per core
TBL = NCORES * NP          # 51200 rows in the gathered tables
PAD_PAIR = 3199            # core-0 pad pair (rows 6398/6399, all-zero)
F32 = mybir.dt.float32
BF16 = mybir.dt.bfloat16
I16 = mybir.dt.int16
BF = ml_dtypes.bfloat16

_TRACE = False
_LAST = None               # BassKernelResults of the most recent run


def _wrap16(flat_idx):
    """dma_gather index layout: element i at [i%16, i//16], replicated to
    128 partitions (one copy per GpSimd core)."""
    n = len(flat_idx)
    a = np.zeros((16, n // 16), np.int16)
    i = np.arange(n)
    a[i % 16, i // 16] = flat_idx.astype(np.int16)
    return np.tile(a, (8, 1))


def _host_prep(x, edge_index, W1, b1, W2, b2):
    src = np.asarray(edge_index[0], dtype=np.int64)
    dst = np.asarray(edge_index[1], dtype=np.int64)
    x = np.asarray(x, dtype=np.float32)

    deg = np.bincount(dst, minlength=N) + 1          # with self-loops
    order = np.argsort(-deg, kind="stable")          # rank -> node id
    r = np.arange(N)
    perm = np.empty(N, np.int64)                     # perm[c*NSH+pos] = node
    pos_of = np.empty(N, np.int64)                   # node -> c*NP + pos
    perm[(r % NCORES) * NSH + r // NCORES] = order
    pos_of[order] = (r % NCORES) * NP + r // NCORES

    all_src = np.concatenate([src, np.arange(N, dtype=np.int64)])
    all_dst = np.concatenate([dst, np.arange(N, dtype=np.int64)])
    dpos = pos_of[all_dst]
    spos = pos_of[all_src]

    # single window: idx = pair row (spos//2) < 25600, parity mask selects
    # the odd half at consume time. Pads point at core-0's zero pad pair.
    o = np.argsort(dpos, kind="stable")
    sd = dpos[o]
    ss = spos[o]
    st = np.searchsorted(sd, np.arange(TBL))
    j = np.arange(len(sd)) - st[sd]
    cnt = np.bincount(dpos, minlength=TBL)
    Kct = cnt.reshape(NCORES, NT, 128).max(axis=2)
    K0 = Kct.max(axis=0).astype(np.int64)
    K1 = np.zeros(NT, np.int64)
    offs0 = np.concatenate([[0], np.cumsum(K0)]).astype(np.int64)
    idx0 = np.full((NCORES, 128, int(offs0[-1])), PAD_PAIR, dtype=np.int64)
    msk0 = np.zeros((NCORES, 128, int(offs0[-1])), dtype=np.uint8)
    c_of = sd // NP
    lp = sd % NP
    idx0[c_of, lp % 128, offs0[lp // 128] + j] = ss // 2
    msk0[c_of, lp % 128, offs0[lp // 128] + j] = ss % 2

    def wrap_core(idx_c, K, offs):
        blocks = []
        for t in range(NT):
            if K[t] == 0:
                continue
            blk = idx_c[:, offs[t]:offs[t + 1]]       # [128, K[t]]
            flat = blk.T.reshape(-1)                  # i = j*128 + p
            blocks.append(_wrap16(flat))
        if not blocks:
            return np.zeros((128, 1), np.int16)
        return np.ascontiguousarray(np.concatenate(blocks, axis=1))

    def mask_core(msk_c, K, offs):
        cols = [msk_c[:, offs[t]:offs[t + 1]] for t in range(NT) if K[t] > 0]
        return np.ascontiguousarray(np.concatenate(cols, axis=1))

    in_maps = []
    for c in range(NCORES):
        pc = perm[c * NSH:(c + 1) * NSH]
        xp = np.zeros((NP, F0), dtype=np.float32)
        xp[:NSH] = x[pc]
        xt4 = np.ascontiguousarray(xp.T.reshape(4, 128, NP)).astype(BF)
        degp = np.ones(NP, dtype=np.float32)
        degp[:NSH] = deg[pc].astype(np.float32)
        degT = np.ascontiguousarray(degp.reshape(NT, 128).T)  # [128, NT]
        m = {
            "xt4": xt4,
            "w1r": np.ascontiguousarray(
                np.asarray(W1, np.float32).reshape(4, 128, F1)).astype(BF),
            "w2": np.ascontiguousarray(np.asarray(W2, np.float32)).astype(BF),
            "degT": degT,
            "idx0": wrap_core(idx0[c], K0, offs0),
            "mskp": mask_core(msk0[c], K0, offs0),
        }
        in_maps.append(m)

    return in_maps, perm, K0, K1


def _build(K0, K1, w0n, w1n, b1_zero, b2_zero):
    Relu = mybir.ActivationFunctionType.Relu
    Copy = mybir.ActivationFunctionType.Copy
    Sqrt = mybir.ActivationFunctionType.Sqrt

    nc = bacc.Bacc("TRN2", target_bir_lowering=False, num_devices=NCORES)

    xt4_d = nc.dram_tensor("xt4", [4, 128, NP], BF16, kind="ExternalInput")
    w1r_d = nc.dram_tensor("w1r", [4, 128, F1], BF16, kind="ExternalInput")
    w2_d = nc.dram_tensor("w2", [F1, F2], BF16, kind="ExternalInput")
    degT_d = nc.dram_tensor("degT", [128, NT], F32, kind="ExternalInput")
    i0_d = nc.dram_tensor("idx0", [128, w0n], I16, kind="ExternalInput")
    mk_d = nc.dram_tensor("mskp", [128, w1n], mybir.dt.uint8,
                          kind="ExternalInput")
    b1r_d = b2r_d = None
    if not b1_zero:
        b1r_d = nc.dram_tensor("b1r", [128, F1], F32, kind="ExternalInput")
    if not b2_zero:
        b2r_d = nc.dram_tensor("b2r", [128, F2], F32, kind="ExternalInput")
    out_d = nc.dram_tensor("out", [NP, F2], F32, kind="ExternalOutput")

    h1_loc = nc.dram_tensor("h1_loc", [NP, F1], BF16, kind="Internal")
    h1_full = nc.dram_tensor("h1_full", [TBL + 2, F1], BF16, kind="Internal",
                             addr_space="Shared")
    h2_loc = nc.dram_tensor("h2_loc", [NP, 128], BF16, kind="Internal")
    h2_full = nc.dram_tensor("h2_full", [TBL + 2, 128], BF16, kind="Internal",
                             addr_space="Shared")

    rg = [list(range(NCORES))]
    K = (np.asarray(K0) + np.asarray(K1)).astype(np.int64)
    KMAX = int(K.max())
    GKMAX = int(max(K[t0:t0 + 2].sum() for t0 in range(0, NT, 2)))
    GROUPS = []
    for t0 in range(0, NT, 2):
        g = [t for t in range(t0, min(t0 + 2, NT)) if int(K[t]) > 0]
        if g:
            GROUPS.append(g)

    with tile.TileContext(nc, num_cores=NCORES) as tc:
        with (
            tc.tile_pool(name="const", bufs=1) as cpool,
            tc.tile_pool(name="out1", bufs=1) as o1pool,
            tc.tile_pool(name="stream", bufs=3) as spool,
            tc.tile_pool(name="msg", bufs=2) as mpool,
            tc.tile_pool(name="psA", bufs=2, space="PSUM") as psA,
            tc.tile_pool(name="psE", bufs=1, space="PSUM") as psE,
            tc.tile_pool(name="psG", bufs=2, space="PSUM") as psG,
        ):
            # ---- constants -------------------------------------------------
            w1sb = cpool.tile([128, 4, F1], BF16)
            nc.sync.dma_start(out=w1sb[:], in_=w1r_d[:].rearrange("k p f -> p k f"))
            w2sb = cpool.tile([128, F2], BF16)
            nc.sync.dma_start(out=w2sb[:], in_=w2_d[:])
            degsb = cpool.tile([128, NT], F32)
            nc.sync.dma_start(out=degsb[:], in_=degT_d[:])
            i0sb = cpool.tile([128, w0n], I16)
            nc.sync.dma_start(out=i0sb[:], in_=i0_d[:])
            mksb = cpool.tile([128, w1n], mybir.dt.uint8)
            nc.sync.dma_start(out=mksb[:], in_=mk_d[:])
            identf = cpool.tile([128, 128], F32)
            make_identity(nc, identf[:])
            identb = cpool.tile([128, 128], BF16)
            nc.scalar.copy(identb[:], identf[:])
            b1sb = b2sb = None
            if not b1_zero:
                b1sb = cpool.tile([128, F1], F32)
                nc.sync.dma_start(out=b1sb[:], in_=b1r_d[:])
            if not b2_zero:
                b2sb = cpool.tile([128, F2], F32)
                nc.sync.dma_start(out=b2sb[:], in_=b2r_d[:])

            rec = cpool.tile([128, NT], F32)
            nc.vector.reciprocal(rec[:], degsb[:])
            dinv = cpool.tile([128, NT], F32)
            nc.scalar.activation(dinv[:], rec[:], Sqrt)

            out1 = o1pool.tile([128, NT, F1], BF16)

            # ---- phase B: h1 = (x @ W1) * dinv -----------------------------
            for t in range(NT):
                xt = spool.tile([128, 4, 128], BF16, tag="xt")
                nc.sync.dma_start(
                    out=xt[:],
                    in_=xt4_d[:, :, ts(t, 128)].rearrange("k p n -> p k n"))
                ph = psA.tile([128, F1], F32, tag="ph")
                for k in range(4):
                    nc.tensor.matmul(ph[:], lhsT=xt[:, k, :], rhs=w1sb[:, k, :],
                                     start=(k == 0), stop=(k == 3))
                h1t = spool.tile([128, F1], BF16, tag="h1t")
                nc.scalar.activation(h1t[:], ph[:], Copy, scale=dinv[:, t:t + 1])
                nc.sync.dma_start(out=h1_loc[ts(t, 128), :], in_=h1t[:])

            # ---- phase C: AllGather h1 -------------------------------------
            nc.gpsimd.collective_compute(
                "AllGather", mybir.AluOpType.bypass, replica_groups=rg,
                ins=[h1_loc[:]], outs=[h1_full[0:TBL, :]])

            # ---- pair-row gather AP: idx k fetches table rows [2k, 2k+1] ---
            def pair_ap(table, row_elems):
                a = table[0:TBL, :]
                return bass.AP(a.tensor, a.offset,
                               [[2 * row_elems, TBL // 2], [1, 2 * row_elems]])

            # ---- aggregation helper (G tiles batched per gather pair) ------
            def aggregate(table, row_elems, feat, ps_pool, ps_tag, consume):
                o0 = om = 0
                ap0 = pair_ap(table, row_elems)
                for g in GROUPS:
                    kg = sum(int(K[t]) for t in g)
                    msg = mpool.tile([128, GKMAX, 2 * row_elems], BF16,
                                     tag="msg")
                    n0 = 128 * kg
                    nc.gpsimd.dma_gather(
                        out_ap=msg[:, :kg, :], in_ap=ap0,
                        idxs_ap=i0sb[:, o0:o0 + n0 // 16],
                        num_idxs=n0, num_idxs_reg=n0,
                        elem_size=2 * row_elems, elem_step=2 * row_elems,
                        single_packet=False)
                    o0 += n0 // 16
                    mka = mksb[:, om:om + kg]
                    mask_b = bass.AP(mka.tensor, mka.offset,
                                     [mka.ap[0], mka.ap[1], [0, feat]])
                    nc.vector.copy_predicated(
                        msg[:, :kg, 0:feat], mask_b,
                        msg[:, :kg, 2 * row_elems - row_elems:
                            2 * row_elems - row_elems + feat])
                    om += kg
                    c0 = 0
                    for t in g:
                        kt = int(K[t])
                        po = ps_pool.tile([128, feat], F32, tag=ps_tag)
                        for j in range(kt):
                            nc.tensor.matmul(po[:], lhsT=identb[:],
                                             rhs=msg[:, c0 + j, 0:feat],
                                             start=(j == 0),
                                             stop=(j == kt - 1))
                        c0 += kt
                        consume(t, po)

            # ---- phase D+E: out1 = relu(dinv*Agg(h1)); h2 = (out1@W2)*dinv -
            def consume1(t, po):
                if b1_zero:
                    nc.scalar.activation(out1[:, t, :], po[:], Relu,
                                         scale=dinv[:, t:t + 1])
                else:
                    tmp = spool.tile([128, F1], F32, tag="tmp1")
                    nc.scalar.activation(tmp[:], po[:], Copy,
                                         scale=dinv[:, t:t + 1])
                    tmp2 = spool.tile([128, F1], F32, tag="tmp2")
                    nc.vector.tensor_tensor(out=tmp2[:], in0=tmp[:],
                                            in1=b1sb[:],
                                            op=mybir.AluOpType.add)
                    nc.scalar.activation(out1[:, t, :], tmp2[:], Relu)
                phase_e(t)

            def phase_e(t):
                pT = psE.tile([128, F1], BF16, tag="pT")
                nc.tensor.transpose(pT[:], out1[:, t, :], identb[:])
                o1T = spool.tile([128, F1], BF16, tag="o1T")
                nc.scalar.copy(o1T[:], pT[:])
                ph2 = psE.tile([128, F2], F32, tag="ph2")
                nc.tensor.matmul(ph2[:], lhsT=o1T[:], rhs=w2sb[:],
                                 start=True, stop=True)
                h2t = spool.tile([128, 128], BF16, tag="h2t")
                nc.vector.memset(h2t[:, F2:128], 0.0)
                nc.scalar.activation(h2t[:, 0:F2], ph2[:], Copy,
                                     scale=dinv[:, t:t + 1])
                nc.sync.dma_start(out=h2_loc[ts(t, 128), :], in_=h2t[:])

            # zero-degree (all-pad) tiles still need zero h2 rows
            for t in range(NT):
                if int(K[t]) == 0:
                    nc.vector.memset(out1[:, t, :], 0.0)
                    phase_e(t)

            aggregate(h1_full, F1, F1, psA, "po", consume1)

            # ---- phase F: AllGather h2 -------------------------------------
            nc.gpsimd.collective_compute(
                "AllGather", mybir.AluOpType.bypass, replica_groups=rg,
                ins=[h2_loc[:]], outs=[h2_full[0:TBL, :]])

            # ---- phase G: out = dinv * Agg(h2) + b2 ------------------------
            def consume2(t, po):
                o2t = spool.tile([128, F2], F32, tag="o2t")
                nc.scalar.activation(o2t[:], po[:], Copy,
                                     scale=dinv[:, t:t + 1])
                if not b2_zero:
                    nc.vector.tensor_tensor(out=o2t[:], in0=o2t[:],
                                            in1=b2sb[:],
                                            op=mybir.AluOpType.add)
                nc.sync.dma_start(out=out_d[ts(t, 128), :], in_=o2t[:])

            aggregate(h2_full, 128, F2, psG, "po2", consume2)

    nc.compile()
    return nc


def kernel(x, edge_index, W1, b1, W2, b2):
    global _LAST
    b1 = np.asarray(b1, np.float32)
    b2 = np.asarray(b2, np.float32)
    in_maps, perm, K0, K1 = _host_prep(x, edge_index, W1, b1, W2, b2)

    b1_zero = bool(np.all(b1 == 0))
    b2_zero = bool(np.all(b2 == 0))
    if not b1_zero:
        for m in in_maps:
            m["b1r"] = np.ascontiguousarray(np.tile(b1[None, :], (128, 1)))
    if not b2_zero:
        for m in in_maps:
            m["b2r"] = np.ascontiguousarray(np.tile(b2[None, :], (128, 1)))

    w0n = in_maps[0]["idx0"].shape[1]
    w1n = in_maps[0]["mskp"].shape[1]
    nc = _build(K0, K1, w0n, w1n, b1_zero, b2_zero)

    res = bass_utils.run_bass_kernel_spmd(
        nc, in_maps, core_ids=list(range(NCORES)), trace=_TRACE)
    _LAST = res

    out = np.empty((N, F2), dtype=np.float32)
    for c in range(NCORES):
        pc = perm[c * NSH:(c + 1) * NSH]
        out[pc] = res.results[c]["out"][:NSH]
    return out


# revision 14
# speedup vs baseline: 1.3140x; 1.2534x over previous
"""GCN (2-layer, PyG GCNConv semantics) on 8 Trainium2 NeuronCores.

Strategy (graph/data parallel, destination-bucketed, gather-based):
  - Nodes sorted by in-degree (desc) and dealt round-robin to the 8
    cores (6250 real + 150 pad each, 50 dest tiles of 128). Sorting
    makes each 128-node dest tile near-uniform in degree, so per-tile
    chunk counts K[t] (shared across cores, SPMD) are tight.
  - Normalization factored per-node: tables hold dinv[v]*h[v]; the
    aggregation is an unweighted sum; results are post-scaled by
    dinv[dst]. No per-edge multiplies.
  - Aggregation: per dest tile, dma_gather fetches, for each (dest
    partition p, chunk j) slot, the table rows [v, v+1] where v is the
    j-th in-neighbor of p (elem_size = 2 rows, elem_step = 1 row).
    Only the first half is consumed by the identity-matmul PSUM
    accumulation (partition index == destination, so scatter is free);
    the second row is don't-care. Indexing by node row keeps int16
    indices valid inside two OVERLAPPING 32768-row windows (bases 0
    and 18432); overlap-region edges are assigned to whichever window
    balances per-dest counts, minimizing padding. Pad slots point at
    guaranteed-zero pad rows. The same index arrays serve both layers.
  - Tables are bf16 (512B gather rows); accumulation stays fp32 in
    PSUM. Layer-2 features are padded 64->128 cols so the row stride
    meets the gather engine's 256B-multiple requirement.
  - Transformed tables are AllGathered (halo exchange) so every core
    gathers from a local full table; W1/W2 replicated.
"""

import numpy as np
import ml_dtypes

import concourse.bacc as bacc
import concourse.bass as bass
import concourse.mybir as mybir
import concourse.tile as tile
from concourse import bass_utils
from concourse.bass import ts
from concourse.masks import make_identity

N = 50000
F0, F1, F2 = 512, 128, 64
NCORES = 8
NSH = N // NCORES          # 6250 real nodes per core
NP = 6400                  # padded nodes per core (50 tiles of 128)
NT = NP // 128             # 50 dest tiles per core
TBL = NCORES * NP          # 51200 rows in the gathered tables
PAD_PAIR = 3199            # core-0 pad pair (rows 6398/6399, all-zero)
F32 = mybir.dt.float32
BF16 = mybir.dt.bfloat16
I16 = mybir.dt.int16
BF = ml_dtypes.bfloat16

_TRACE = False
_LAST = None               # BassKernelResults of the most recent run


def _wrap16(flat_idx):
    """dma_gather index layout: element i at [i%16, i//16], replicated to
    128 partitions (one copy per GpSimd core)."""
    n = len(flat_idx)
    a = np.zeros((16, n // 16), np.int16)
    i = np.arange(n)
    a[i % 16, i // 16] = flat_idx.astype(np.int16)
    return np.tile(a, (8, 1))


def _host_prep(x, edge_index, W1, b1, W2, b2):
    src = np.asarray(edge_index[0], dtype=np.int64)
    dst = np.asarray(edge_index[1], dtype=np.int64)
    x = np.asarray(x, dtype=np.float32)

    deg = np.bincount(dst, minlength=N) + 1          # with self-loops
    order = np.argsort(-deg, kind="stable")          # rank -> node id
    r = np.arange(N)
    perm = np.empty(N, np.int64)                     # perm[c*NSH+pos] = node
    pos_of = np.empty(N, np.int64)                   # node -> c*NP + pos
    perm[(r % NCORES) * NSH + r // NCORES] = order
    pos_of[order] = (r % NCORES) * NP + r // NCORES

    all_src = np.concatenate([src, np.arange(N, dtype=np.int64)])
    all_dst = np.concatenate([dst, np.arange(N, dtype=np.int64)])
    dpos = pos_of[all_dst]
    spos = pos_of[all_src]

    # single window: idx = pair row (spos//2) < 25600, parity mask selects
    # the odd half at consume time. Pads point at core-0's zero pad pair.
    o = np.argsort(dpos, kind="stable")
    sd = dpos[o]
    ss = spos[o]
    st = np.searchsorted(sd, np.arange(TBL))
    j = np.arange(len(sd)) - st[sd]
    cnt = np.bincount(dpos, minlength=TBL)
    Kct = cnt.reshape(NCORES, NT, 128).max(axis=2)
    K0 = Kct.max(axis=0).astype(np.int64)
    K1 = np.zeros(NT, np.int64)
    offs0 = np.concatenate([[0], np.cumsum(K0)]).astype(np.int64)
    idx0 = np.full((NCORES, 128, int(offs0[-1])), PAD_PAIR, dtype=np.int64)
    msk0 = np.zeros((NCORES, 128, int(offs0[-1])), dtype=np.uint8)
    c_of = sd // NP
    lp = sd % NP
    idx0[c_of, lp % 128, offs0[lp // 128] + j] = ss // 2
    msk0[c_of, lp % 128, offs0[lp // 128] + j] = ss % 2

    def wrap_core(idx_c, K, offs):
        blocks = []
        for t in range(NT):
            if K[t] == 0:
                continue
            blk = idx_c[:, offs[t]:offs[t + 1]]       # [128, K[t]]
            flat = blk.T.reshape(-1)                  # i = j*128 + p
            blocks.append(_wrap16(flat))
        if not blocks:
            return np.zeros((128, 1), np.int16)
        return np.ascontiguousarray(np.concatenate(blocks, axis=1))

    def mask_core(msk_c, K, offs):
        cols = [msk_c[:, offs[t]:offs[t + 1]] for t in range(NT) if K[t] > 0]
        return np.ascontiguousarray(np.concatenate(cols, axis=1))

    in_maps = []
    for c in range(NCORES):
        pc = perm[c * NSH:(c + 1) * NSH]
        xp = np.zeros((NP, F0), dtype=np.float32)
        xp[:NSH] = x[pc]
        xt4 = np.ascontiguousarray(xp.T.reshape(4, 128, NP)).astype(BF)
        degp = np.ones(NP, dtype=np.float32)
        degp[:NSH] = deg[pc].astype(np.float32)
        degT = np.ascontiguousarray(degp.reshape(NT, 128).T)  # [128, NT]
        m = {
            "xt4": xt4,
            "w1r": np.ascontiguousarray(
                np.asarray(W1, np.float32).reshape(4, 128, F1)).astype(BF),
            "w2": np.ascontiguousarray(np.asarray(W2, np.float32)).astype(BF),
            "degT": degT,
            "idx0": wrap_core(idx0[c], K0, offs0),
            "mskp": mask_core(msk0[c], K0, offs0),
        }
        in_maps.append(m)

    return in_maps, perm, K0, K1


def _build(K0, K1, w0n, w1n, b1_zero, b2_zero):
    Relu = mybir.ActivationFunctionType.Relu
    Copy = mybir.ActivationFunctionType.Copy
    Sqrt = mybir.ActivationFunctionType.Sqrt

    nc = bacc.Bacc("TRN2", target_bir_lowering=False, num_devices=NCORES)

    xt4_d = nc.dram_tensor("xt4", [4, 128, NP], BF16, kind="ExternalInput")
    w1r_d = nc.dram_tensor("w1r", [4, 128, F1], BF16, kind="ExternalInput")
    w2_d = nc.dram_tensor("w2", [F1, F2], BF16, kind="ExternalInput")
    degT_d = nc.dram_tensor("degT", [128, NT], F32, kind="ExternalInput")
    i0_d = nc.dram_tensor("idx0", [128, w0n], I16, kind="ExternalInput")
    mk_d = nc.dram_tensor("mskp", [128, w1n], mybir.dt.uint8,
                          kind="ExternalInput")
    b1r_d = b2r_d = None
    if not b1_zero:
        b1r_d = nc.dram_tensor("b1r", [128, F1], F32, kind="ExternalInput")
    if not b2_zero:
        b2r_d = nc.dram_tensor("b2r", [128, F2], F32, kind="ExternalInput")
    out_d = nc.dram_tensor("out", [NP, F2], F32, kind="ExternalOutput")

    h1_loc = nc.dram_tensor("h1_loc", [NP, F1], BF16, kind="Internal")
    h1_full = nc.dram_tensor("h1_full", [TBL + 2, F1], BF16, kind="Internal",
                             addr_space="Shared")
    h2_loc = nc.dram_tensor("h2_loc", [NP, 128], BF16, kind="Internal")
    h2_full = nc.dram_tensor("h2_full", [TBL + 2, 128], BF16, kind="Internal",
                             addr_space="Shared")

    rg = [list(range(NCORES))]
    K = (np.asarray(K0) + np.asarray(K1)).astype(np.int64)
    KMAX = int(K.max())
    GKMAX = int(max(K[t0:t0 + 2].sum() for t0 in range(0, NT, 2)))
    GROUPS = []
    for t0 in range(0, NT, 2):
        g = [t for t in range(t0, min(t0 + 2, NT)) if int(K[t]) > 0]
        if g:
            GROUPS.append(g)

    with tile.TileContext(nc, num_cores=NCORES) as tc:
        with (
            tc.tile_pool(name="const", bufs=1) as cpool,
            tc.tile_pool(name="out1", bufs=1) as o1pool,
            tc.tile_pool(name="stream", bufs=3) as spool,
            tc.tile_pool(name="msg", bufs=2) as mpool,
            tc.tile_pool(name="psA", bufs=2, space="PSUM") as psA,
            tc.tile_pool(name="psE", bufs=1, space="PSUM") as psE,
            tc.tile_pool(name="psG", bufs=2, space="PSUM") as psG,
        ):
            # ---- constants -------------------------------------------------
            w1sb = cpool.tile([128, 4, F1], BF16)
            nc.sync.dma_start(out=w1sb[:], in_=w1r_d[:].rearrange("k p f -> p k f"))
            w2sb = cpool.tile([128, F2], BF16)
            nc.sync.dma_start(out=w2sb[:], in_=w2_d[:])
            degsb = cpool.tile([128, NT], F32)
            nc.sync.dma_start(out=degsb[:], in_=degT_d[:])
            i0sb = cpool.tile([128, w0n], I16)
            nc.sync.dma_start(out=i0sb[:], in_=i0_d[:])
            mksb = cpool.tile([128, w1n], mybir.dt.uint8)
            nc.sync.dma_start(out=mksb[:], in_=mk_d[:])
            identf = cpool.tile([128, 128], F32)
            make_identity(nc, identf[:])
            identb = cpool.tile([128, 128], BF16)
            nc.scalar.copy(identb[:], identf[:])
            b1sb = b2sb = None
            if not b1_zero:
                b1sb = cpool.tile([128, F1], F32)
                nc.sync.dma_start(out=b1sb[:], in_=b1r_d[:])
            if not b2_zero:
                b2sb = cpool.tile([128, F2], F32)
                nc.sync.dma_start(out=b2sb[:], in_=b2r_d[:])

            rec = cpool.tile([128, NT], F32)
            nc.vector.reciprocal(rec[:], degsb[:])
            dinv = cpool.tile([128, NT], F32)
            nc.scalar.activation(dinv[:], rec[:], Sqrt)

            out1 = o1pool.tile([128, NT, F1], BF16)

            # ---- phase B: h1 = (x @ W1) * dinv -----------------------------
            for t in range(NT):
                xt = spool.tile([128, 4, 128], BF16, tag="xt")
                nc.sync.dma_start(
                    out=xt[:],
                    in_=xt4_d[:, :, ts(t, 128)].rearrange("k p n -> p k n"))
                ph = psA.tile([128, F1], F32, tag="ph")
                for k in range(4):
                    nc.tensor.matmul(ph[:], lhsT=xt[:, k, :], rhs=w1sb[:, k, :],
                                     start=(k == 0), stop=(k == 3))
                h1t = spool.tile([128, F1], BF16, tag="h1t")
                nc.scalar.activation(h1t[:], ph[:], Copy, scale=dinv[:, t:t + 1])
                nc.sync.dma_start(out=h1_loc[ts(t, 128), :], in_=h1t[:])

            # ---- phase C: AllGather h1 -------------------------------------
            nc.gpsimd.collective_compute(
                "AllGather", mybir.AluOpType.bypass, replica_groups=rg,
                ins=[h1_loc[:]], outs=[h1_full[0:TBL, :]])

            # ---- pair-row gather AP: idx k fetches table rows [2k, 2k+1] ---
            def pair_ap(table, row_elems):
                a = table[0:TBL, :]
                return bass.AP(a.tensor, a.offset,
                               [[2 * row_elems, TBL // 2], [1, 2 * row_elems]])

            # ---- aggregation helper (G tiles batched per gather pair) ------
            def aggregate(table, row_elems, feat, ps_pool, ps_tag, consume):
                o0 = om = 0
                ap0 = pair_ap(table, row_elems)
                for g in GROUPS:
                    kg = sum(int(K[t]) for t in g)
                    msg = mpool.tile([128, GKMAX, 2 * row_elems], BF16,
                                     tag="msg")
                    ka = kg // 2
                    for (lo, hi) in ((0, ka), (ka, kg)):
                        kk = hi - lo
                        if kk == 0:
                            continue
                        nn = 128 * kk
                        nc.gpsimd.dma_gather(
                            out_ap=msg[:, lo:hi, :], in_ap=ap0,
                            idxs_ap=i0sb[:, o0:o0 + nn // 16],
                            num_idxs=nn, num_idxs_reg=nn,
                            elem_size=2 * row_elems, elem_step=2 * row_elems,
                            single_packet=False)
                        o0 += nn // 16
                    mka = mksb[:, om:om + kg]
                    mask_b = bass.AP(mka.tensor, mka.offset,
                                     [mka.ap[0], mka.ap[1], [0, feat]])
                    nc.vector.copy_predicated(
                        msg[:, :kg, 0:feat], mask_b,
                        msg[:, :kg, 2 * row_elems - row_elems:
                            2 * row_elems - row_elems + feat])
                    om += kg
                    c0 = 0
                    for t in g:
                        kt = int(K[t])
                        po = ps_pool.tile([128, feat], F32, tag=ps_tag)
                        for j in range(kt):
                            nc.tensor.matmul(po[:], lhsT=identb[:],
                                             rhs=msg[:, c0 + j, 0:feat],
                                             start=(j == 0),
                                             stop=(j == kt - 1))
                        c0 += kt
                        consume(t, po)

            # ---- phase D+E: out1 = relu(dinv*Agg(h1)); h2 = (out1@W2)*dinv -
            def consume1(t, po):
                if b1_zero:
                    nc.scalar.activation(out1[:, t, :], po[:], Relu,
                                         scale=dinv[:, t:t + 1])
                else:
                    tmp = spool.tile([128, F1], F32, tag="tmp1")
                    nc.scalar.activation(tmp[:], po[:], Copy,
                                         scale=dinv[:, t:t + 1])
                    tmp2 = spool.tile([128, F1], F32, tag="tmp2")
                    nc.vector.tensor_tensor(out=tmp2[:], in0=tmp[:],
                                            in1=b1sb[:],
                                            op=mybir.AluOpType.add)
                    nc.scalar.activation(out1[:, t, :], tmp2[:], Relu)
                phase_e(t)

            def phase_e(t):
                pT = psE.tile([128, F1], BF16, tag="pT")
                nc.tensor.transpose(pT[:], out1[:, t, :], identb[:])
                o1T = spool.tile([128, F1], BF16, tag="o1T")
                nc.scalar.copy(o1T[:], pT[:])
                ph2 = psE.tile([128, F2], F32, tag="ph2")
                nc.tensor.matmul(ph2[:], lhsT=o1T[:], rhs=w2sb[:],
                                 start=True, stop=True)
                h2t = spool.tile([128, 128], BF16, tag="h2t")
                nc.vector.memset(h2t[:, F2:128], 0.0)
                nc.scalar.activation(h2t[:, 0:F2], ph2[:], Copy,
                                     scale=dinv[:, t:t + 1])
                nc.sync.dma_start(out=h2_loc[ts(t, 128), :], in_=h2t[:])

            # zero-degree (all-pad) tiles still need zero h2 rows
            for t in range(NT):
                if int(K[t]) == 0:
                    nc.vector.memset(out1[:, t, :], 0.0)
                    phase_e(t)

            aggregate(h1_full, F1, F1, psA, "po", consume1)

            # ---- phase F: AllGather h2 -------------------------------------
            nc.gpsimd.collective_compute(
                "AllGather", mybir.AluOpType.bypass, replica_groups=rg,
                ins=[h2_loc[:]], outs=[h2_full[0:TBL, :]])

            # ---- phase G: out = dinv * Agg(h2) + b2 ------------------------
            def consume2(t, po):
                o2t = spool.tile([128, F2], F32, tag="o2t")
                nc.scalar.activation(o2t[:], po[:], Copy,
                                     scale=dinv[:, t:t + 1])
                if not b2_zero:
                    nc.vector.tensor_tensor(out=o2t[:], in0=o2t[:],
                                            in1=b2sb[:],
                                            op=mybir.AluOpType.add)
                nc.sync.dma_start(out=out_d[ts(t, 128), :], in_=o2t[:])

            aggregate(h2_full, 128, F2, psG, "po2", consume2)

    nc.compile()
    return nc


def kernel(x, edge_index, W1, b1, W2, b2):
    global _LAST
    b1 = np.asarray(b1, np.float32)
    b2 = np.asarray(b2, np.float32)
    in_maps, perm, K0, K1 = _host_prep(x, edge_index, W1, b1, W2, b2)

    b1_zero = bool(np.all(b1 == 0))
    b2_zero = bool(np.all(b2 == 0))
    if not b1_zero:
        for m in in_maps:
            m["b1r"] = np.ascontiguousarray(np.tile(b1[None, :], (128, 1)))
    if not b2_zero:
        for m in in_maps:
            m["b2r"] = np.ascontiguousarray(np.tile(b2[None, :], (128, 1)))

    w0n = in_maps[0]["idx0"].shape[1]
    w1n = in_maps[0]["mskp"].shape[1]
    nc = _build(K0, K1, w0n, w1n, b1_zero, b2_zero)

    res = bass_utils.run_bass_kernel_spmd(
        nc, in_maps, core_ids=list(range(NCORES)), trace=_TRACE)
    _LAST = res

    out = np.empty((N, F2), dtype=np.float32)
    for c in range(NCORES):
        pc = perm[c * NSH:(c + 1) * NSH]
        out[pc] = res.results[c]["out"][:NSH]
    return out
